# revision 9
# baseline (speedup 1.0000x reference)
"""Trainium2 Bass kernel for ParallelLMHeadWithLoRA.

logits = hidden @ W^T + (hidden @ A^T) @ B^T
  hidden [2048, 4096] f32, W [32000, 4096] f32, A [16, 4096], B [32000, 16]

Strategy (8 NeuronCores, tensor-parallel over vocab):
  - Each core owns a 4000-wide vocab slice of W and B (sharding hint),
    split into 32 blocks of 125 columns. (125, not 128: 128-column fp16
    weight loads trigger the 4-XBUS fast-weight-load path, which was
    measured SLOWER here -- it contends with the moving-operand stream.)
  - Host pre-transposes/blocks the operands (fp16) so every DMA is a
    contiguous slab:
      wtb[vb, p, dc, j] = W[v0 + vb*125 + j, dc*128 + p]   (per-core)
      htt = hidden^T [4096, 2048]                           (replicated)
      atz[p, dc, c] = A[c, dc*128+p] for c<16, 0 for 16<=c<32
      bt4[32j+r, v] = B[v0+v, r], j=0..3, other rows 0      (per-core)
  - Base path (per core): logits^T[v, t] = sum_dc wt[128,125].T @ ht[128,512]
    accumulated over 32 contraction chunks in PSUM.
  - LoRA path, restructured so no matmul wastes the 128-wide array:
     1. ao partials via 4x COLUMN TILING: per 512-token block, rounds
        of 4 concurrent matmuls (tile_position=(0,32j)) compute
        pa[32j:32j+32, t] = sum_{dc=4r+j} atz_dc.T @ ht_dc -- i.e. the
        K=4096 reduction split 4 ways across psum partition groups
        (zero-padded A columns keep all 128 partitions written).
        8 rounds instead of 32 full-array passes: ~4x faster.
     2. pa is copied once to SBUF (pr, fp16). The B-fold then consumes
        the UNREDUCED partials directly: bt4 has B^T replicated at
        partition offsets 0/32/64/96 (zeros elsewhere), so ONE ordinary
        K=128 matmul  bt4.T @ pr  = sum_j B^T.T @ partial_j  performs
        the cross-group reduction and the LoRA fold simultaneously,
        accumulating into the same psum group as the base matmuls.
  - PE warmup: ~24 matmuls on a zeroed scratch tile, alternating two
    psum banks so they pipeline. The first ~10us of the kernel are
    DMA-launch latency with no compute available, and the HAM clock
    gate holds the PE at 1.2GHz until it has been busy ~3.4us; the
    warmup rides out both for free.
  - DMA: ht streams g-major (first 1024 tokens for all dc, then the
    rest) on the sync queue; wt0/wt1 prefetch on the scalar queue and
    atz/wt2/wt3/bt4 on the gpsimd queue so the three streams land in
    parallel during the ramp.
  - hidden^T is fp16 so all 2048 tokens stay SBUF-resident; W streams
    through exactly once => PE-bound at 1 cycle/row.
  - Output logits^T stored fp16 (|logits| <~ 8, fp16 rounding ~5e-4
    relative); host upcasts to fp32. Halves write traffic + tail.
"""

import numpy as np

import concourse.mybir as mybir
import concourse.tile as tile
from concourse import bacc
from concourse.bass_utils import run_bass_kernel_spmd

P = 128
N_TOK = 2048
D = 4096
V = 32000
R = 16
NCORES = 8

VC = V // NCORES          # 4000 vocab per core
VCP = VC
VBS = 125                 # vocab block (psum partition dim)
VB = VCP // VBS           # 32 vocab blocks
DC = D // P               # 32 contraction chunks
TBS = 512                 # moving free dim per matmul (ISA cap)
TB = N_TOK // TBS         # 4 token blocks
AZ = 32                   # zero-padded A stationary width per col tile

F32 = mybir.dt.float32
F16 = mybir.dt.float16

N_WARM = 24


def build_nc(ht_bufs=2 * DC, wt_bufs=4, out_bufs=8, ps_bufs=7):
    nc = bacc.Bacc(None, target_bir_lowering=False, debug=False)

    wtb = nc.dram_tensor("wtb", [VB, P, DC, VBS], F16, kind="ExternalInput")
    htt = nc.dram_tensor("htt", [D, N_TOK], F16, kind="ExternalInput")
    atz = nc.dram_tensor("atz", [P, DC, AZ], F16, kind="ExternalInput")
    bt4 = nc.dram_tensor("bt4", [P, VCP], F16, kind="ExternalInput")
    outt = nc.dram_tensor("outt", [VCP, N_TOK], F16, kind="ExternalOutput")

    with tile.TileContext(nc) as tc:
        with (
            tc.tile_pool(name="const", bufs=1) as const,
            tc.tile_pool(name="htp", bufs=ht_bufs) as htp,
            tc.tile_pool(name="wtp", bufs=wt_bufs) as wtp,
            tc.tile_pool(name="outp", bufs=out_bufs) as outp,
            tc.tile_pool(name="psp", bufs=ps_bufs, space="PSUM") as psp,
            tc.tile_pool(name="aops", bufs=1, space="PSUM") as aops,
        ):
            # PE warmup (see module docstring). Two alternating banks so
            # consecutive start=True matmuls pipeline instead of
            # serializing on one bank's drain.
            warm = const.tile([P, TBS], F16, name="warm")
            nc.vector.memset(warm[:], 0.0)
            pw = [
                psp.tile([P, TBS], F32, name=f"pw{i}", tag="ps")
                for i in range(2)
            ]
            for i in range(N_WARM):
                nc.tensor.matmul(
                    pw[i % 2][:], warm[:, 0:P], warm[:], start=True, stop=True
                )

            # weight-side DMAs: wt0/wt1 on the scalar queue, the rest on
            # gpsimd, so neither blocks the ht stream on sync.
            wt_tiles = {}
            for vb in range(2):
                wt_t = wtp.tile([P, DC, VBS], F16, name="wt_t", tag="wt")
                nc.scalar.dma_start(wt_t[:], wtb[vb, :, :, :])
                wt_tiles[vb] = wt_t
            at_t = const.tile([P, DC, AZ], F16, name="at_t")
            nc.gpsimd.dma_start(at_t[:], atz[:, :, :])
            for vb in range(2, 4):
                wt_t = wtp.tile([P, DC, VBS], F16, name="wt_t", tag="wt")
                nc.gpsimd.dma_start(wt_t[:], wtb[vb, :, :, :])
                wt_tiles[vb] = wt_t
            bt_t = const.tile([P, VCP], F16, name="bt_t")
            nc.gpsimd.dma_start(bt_t[:], bt4[:, :])

            # resident hidden^T: 64 tiles of [128, 1024] fp16, g-major
            ht_tiles = {}
            for g in range(2):
                for dc in range(DC):
                    ht_t = htp.tile([P, N_TOK // 2], F16,
                                    name=f"ht_{dc}_{g}", tag="ht")
                    nc.sync.dma_start(
                        ht_t[:],
                        htt[dc * P:(dc + 1) * P,
                            g * (N_TOK // 2):(g + 1) * (N_TOK // 2)],
                    )
                    ht_tiles[(dc, g)] = ht_t

            def ht_slice(dc, tb):
                g, r = divmod(tb, 2)
                return ht_tiles[(dc, g)][:, r * TBS:(r + 1) * TBS]

            # LoRA ao partials, 4x column-tiled (see module docstring)
            pr_tiles = []
            for tb in range(TB):
                pa = aops.tile([P, TBS], F32, name="pa", tag="pa")
                for rnd in range(DC // 4):
                    for j in range(4):
                        dc = 4 * rnd + j
                        nc.tensor.matmul(
                            pa[32 * j:32 * j + AZ, :],
                            at_t[:, dc, :],
                            ht_slice(dc, tb),
                            start=(rnd == 0),
                            stop=(rnd == DC // 4 - 1),
                            tile_position=(0, 32 * j),
                        )
                pr_t = const.tile([P, TBS], F16, name=f"pr{tb}")
                nc.vector.tensor_copy(pr_t[:], pa[:])
                pr_tiles.append(pr_t)

            for vb in range(VB):
                if vb in wt_tiles:
                    wt_t = wt_tiles.pop(vb)
                else:
                    wt_t = wtp.tile([P, DC, VBS], F16, name="wt_t", tag="wt")
                    nc.gpsimd.dma_start(wt_t[:], wtb[vb, :, :, :])

                pss = [
                    psp.tile([VBS, TBS], F32, name=f"ps{tb}", tag="ps")
                    for tb in range(TB)
                ]
                for dc in range(DC):
                    for tb in range(TB):
                        nc.tensor.matmul(
                            pss[tb][:],
                            wt_t[:, dc, :],
                            ht_slice(dc, tb),
                            start=(dc == 0),
                            stop=False,
                        )
                # reduce-and-fold the LoRA correction: one ordinary
                # K=128 matmul per token block (bt4's zero rows mask the
                # partial-group padding)
                for tb in range(TB):
                    nc.tensor.matmul(
                        pss[tb][:],
                        bt_t[:, vb * VBS:(vb + 1) * VBS],
                        pr_tiles[tb][:],
                        start=False,
                        stop=True,
                    )
                for tb in range(TB):
                    ts0 = tb * TBS
                    ot = outp.tile([VBS, TBS], F16, name="ot", tag="ot")
                    # split evictions across DVE and ACT so psum banks
                    # free ~2x faster at vb boundaries
                    if tb % 2 == 0:
                        nc.vector.tensor_copy(ot[:], pss[tb][:])
                    else:
                        nc.scalar.copy(ot[:], pss[tb][:])
                    nc.scalar.dma_start(
                        outt[vb * VBS:(vb + 1) * VBS, ts0:ts0 + TBS], ot[:]
                    )
    nc.compile()
    return nc


def _prep_inputs(hidden_states, weight, lora_A, lora_B):
    w = np.asarray(weight, dtype=np.float16)
    # [core, vb, j, dc, p] -> [core, vb, p, dc, j]
    wtb_all = np.ascontiguousarray(
        w.reshape(NCORES, VB, VBS, DC, P).transpose(0, 1, 4, 3, 2)
    )
    htt = np.ascontiguousarray(np.asarray(hidden_states, dtype=np.float16).T)
    # A^T blocked [128, 32, 16], zero-padded to 32 columns
    att = np.asarray(lora_A, dtype=np.float16).T.reshape(DC, P, R).transpose(1, 0, 2)
    atz = np.zeros((P, DC, AZ), dtype=np.float16)
    atz[:, :, :R] = att
    # B^T slice replicated at 32-partition offsets, zeros elsewhere
    btt_all = (
        np.asarray(lora_B, dtype=np.float16).reshape(NCORES, VC, R)
        .transpose(0, 2, 1)
    )
    bt4_all = np.zeros((NCORES, P, VCP), dtype=np.float16)
    for j in range(4):
        bt4_all[:, 32 * j:32 * j + R, :] = btt_all
    return [
        {
            "wtb": wtb_all[c],
            "htt": htt,
            "atz": atz,
            "bt4": bt4_all[c],
        }
        for c in range(NCORES)
    ]


def run(hidden_states, weight, lora_A, lora_B, trace=False, **run_kwargs):
    in_maps = _prep_inputs(hidden_states, weight, lora_A, lora_B)
    nc = build_nc()
    res = run_bass_kernel_spmd(
        nc, in_maps, core_ids=list(range(NCORES)), trace=trace, **run_kwargs
    )
    out = np.empty((N_TOK, V), dtype=np.float32)
    for c in range(NCORES):
        out[:, c * VC:(c + 1) * VC] = res.results[c]["outt"].T.astype(np.float32)
    return out, res


def kernel(hidden_states, weight, lora_A, lora_B):
    out, _ = run(hidden_states, weight, lora_A, lora_B, trace=False)
    return out


# revision 13
# speedup vs baseline: 1.0197x; 1.0197x over previous
"""Trainium2 Bass kernel for ParallelLMHeadWithLoRA.

logits = hidden @ W^T + (hidden @ A^T) @ B^T
  hidden [2048, 4096] f32, W [32000, 4096] f32, A [16, 4096], B [32000, 16]

Strategy (8 NeuronCores, tensor-parallel over vocab):
  - Each core owns a 4000-wide vocab slice of W and B (sharding hint),
    split into 32 blocks of 125 columns. (125, not 128: 128-column fp16
    weight loads trigger the 4-XBUS fast-weight-load path, which was
    measured SLOWER here -- it contends with the moving-operand stream.)
  - Host pre-transposes/blocks the operands (fp16) so every DMA is a
    contiguous slab:
      wtb[vb, p, dc, j] = W[v0 + vb*125 + j, dc*128 + p]   (per-core)
      htt = hidden^T [4096, 2048]                           (replicated)
      atz[p, dc, c] = A[c, dc*128+p] for c<16, 0 for 16<=c<32
      bt4[32j+r, v] = B[v0+v, r], j=0..3, other rows 0      (per-core)
  - Base path (per core): logits^T[v, t] = sum_dc wt[128,125].T @ ht[128,512]
    accumulated over 32 contraction chunks in PSUM.
  - LoRA path, restructured so no matmul wastes the 128-wide array:
     1. ao partials via 4x COLUMN TILING: per 512-token block, rounds
        of 4 concurrent matmuls (tile_position=(0,32j)) compute
        pa[32j:32j+32, t] = sum_{dc=4r+j} atz_dc.T @ ht_dc -- i.e. the
        K=4096 reduction split 4 ways across psum partition groups
        (zero-padded A columns keep all 128 partitions written).
        8 rounds instead of 32 full-array passes: ~4x faster.
     2. pa is copied once to SBUF (pr, fp16). The B-fold then consumes
        the UNREDUCED partials directly: bt4 has B^T replicated at
        partition offsets 0/32/64/96 (zeros elsewhere), so ONE ordinary
        K=128 matmul  bt4.T @ pr  = sum_j B^T.T @ partial_j  performs
        the cross-group reduction and the LoRA fold simultaneously,
        accumulating into the same psum group as the base matmuls.
  - PE warmup: ~24 matmuls on a zeroed scratch tile, alternating two
    psum banks so they pipeline. The first ~10us of the kernel are
    DMA-launch latency with no compute available, and the HAM clock
    gate holds the PE at 1.2GHz until it has been busy ~3.4us; the
    warmup rides out both for free.
  - DMA: ht streams g-major (first 1024 tokens for all dc, then the
    rest) on the sync queue; wt0/wt1 prefetch on the scalar queue and
    atz/wt2/wt3/bt4 on the gpsimd queue so the three streams land in
    parallel during the ramp.
  - hidden^T is fp16 so all 2048 tokens stay SBUF-resident; W streams
    through exactly once => PE-bound at 1 cycle/row.
  - Output logits^T stored fp16 (|logits| <~ 8, fp16 rounding ~5e-4
    relative); host upcasts to fp32. Halves write traffic + tail.
"""

import numpy as np

import concourse.mybir as mybir
import concourse.tile as tile
from concourse import bacc
from concourse.bass_utils import run_bass_kernel_spmd

P = 128
N_TOK = 2048
D = 4096
V = 32000
R = 16
NCORES = 8

VC = V // NCORES          # 4000 vocab per core
VCP = VC
VBS = 125                 # vocab block (psum partition dim)
VB = VCP // VBS           # 32 vocab blocks
DC = D // P               # 32 contraction chunks
TBS = 512                 # moving free dim per matmul (ISA cap)
TB = N_TOK // TBS         # 4 token blocks
AZ = 32                   # zero-padded A stationary width per col tile

F32 = mybir.dt.float32
F16 = mybir.dt.float16

N_WARM = 24


def build_nc(ht_bufs=2 * DC, wt_bufs=4, out_bufs=8, ps_bufs=6):
    nc = bacc.Bacc(None, target_bir_lowering=False, debug=False)

    wtb = nc.dram_tensor("wtb", [VB, P, DC, VBS], F16, kind="ExternalInput")
    htt = nc.dram_tensor("htt", [D, N_TOK], F16, kind="ExternalInput")
    atz = nc.dram_tensor("atz", [P, DC, AZ], F16, kind="ExternalInput")
    bt4 = nc.dram_tensor("bt4", [P, VCP], F16, kind="ExternalInput")
    outt = nc.dram_tensor("outt", [VCP, N_TOK], F16, kind="ExternalOutput")

    with tile.TileContext(nc) as tc:
        with (
            tc.tile_pool(name="const", bufs=1) as const,
            tc.tile_pool(name="htp", bufs=ht_bufs) as htp,
            tc.tile_pool(name="wtp", bufs=wt_bufs) as wtp,
            tc.tile_pool(name="outp", bufs=out_bufs) as outp,
            tc.tile_pool(name="psp", bufs=ps_bufs, space="PSUM") as psp,
            tc.tile_pool(name="aops", bufs=2, space="PSUM") as aops,
        ):
            # PE warmup (see module docstring). Two alternating banks so
            # consecutive start=True matmuls pipeline instead of
            # serializing on one bank's drain. Nonzero data: the HAM
            # activity monitor tracks array toggling, and all-zero
            # operands leave the array electrically idle (measured: 24
            # zero-data warmup matmuls failed to lift the clock gate).
            warm = const.tile([P, TBS], F16, name="warm")
            nc.vector.memset(warm[:], 1.0)
            pw = [
                psp.tile([P, TBS], F32, name=f"pw{i}", tag="ps")
                for i in range(2)
            ]
            for i in range(N_WARM):
                nc.tensor.matmul(
                    pw[i % 2][:], warm[:, 0:P], warm[:], start=True, stop=True
                )

            # weight-side DMAs: wt0/wt1 on the scalar queue, the rest on
            # gpsimd, so neither blocks the ht stream on sync.
            wt_tiles = {}
            for vb in range(2):
                wt_t = wtp.tile([P, DC, VBS], F16, name="wt_t", tag="wt")
                nc.scalar.dma_start(wt_t[:], wtb[vb, :, :, :])
                wt_tiles[vb] = wt_t
            at_t = const.tile([P, DC, AZ], F16, name="at_t")
            nc.gpsimd.dma_start(at_t[:], atz[:, :, :])
            for vb in range(2, 4):
                wt_t = wtp.tile([P, DC, VBS], F16, name="wt_t", tag="wt")
                nc.gpsimd.dma_start(wt_t[:], wtb[vb, :, :, :])
                wt_tiles[vb] = wt_t
            bt_t = const.tile([P, VCP], F16, name="bt_t")
            nc.gpsimd.dma_start(bt_t[:], bt4[:, :])

            # resident hidden^T: 64 tiles of [128, 1024] fp16, g-major
            ht_tiles = {}
            for g in range(2):
                for dc in range(DC):
                    ht_t = htp.tile([P, N_TOK // 2], F16,
                                    name=f"ht_{dc}_{g}", tag="ht")
                    nc.sync.dma_start(
                        ht_t[:],
                        htt[dc * P:(dc + 1) * P,
                            g * (N_TOK // 2):(g + 1) * (N_TOK // 2)],
                    )
                    ht_tiles[(dc, g)] = ht_t

            def ht_slice(dc, tb):
                g, r = divmod(tb, 2)
                return ht_tiles[(dc, g)][:, r * TBS:(r + 1) * TBS]

            # ---- compute schedule -----------------------------------
            # The PE executes its instruction queue strictly in program
            # order, so the emission order here IS the schedule, and any
            # instruction waiting on a DMA stalls everything behind it.
            # The ao pass is paced by the arriving ht stream; emitting it
            # as one block would serialize it in front of the base
            # matmuls (measured: ~45us of DMA-paced ao before the first
            # base matmul). Instead, every vb is processed as two
            # (tb-pair) half-groups matching the g-major ht arrival
            # order, and the ao rounds for each g are interleaved with
            # the first three vbs' half-group sweeps so the PE always
            # has base work queued between DMA-paced ao matmuls.

            def get_wt(vb):
                if vb in wt_tiles:
                    return wt_tiles.pop(vb)
                wt_t = wtp.tile([P, DC, VBS], F16, name="wt_t", tag="wt")
                nc.gpsimd.dma_start(wt_t[:], wtb[vb, :, :, :])
                return wt_t

            pss = {}       # (vb, tb) -> psum tile, allocated lazily
            pr_tiles = {}  # tb -> SBUF fp16 partials

            def base_mm(vb, wt_t, dc, tb):
                ps = pss.get((vb, tb))
                if ps is None:
                    ps = psp.tile([VBS, TBS], F32, name=f"ps{tb}", tag="ps")
                    pss[(vb, tb)] = ps
                nc.tensor.matmul(
                    ps[:],
                    wt_t[:, dc, :],
                    ht_slice(dc, tb),
                    start=(dc == 0),
                    stop=False,
                )

            def ao_round(pa, rnd, tb):
                for j in range(4):
                    dc = 4 * rnd + j
                    nc.tensor.matmul(
                        pa[32 * j:32 * j + AZ, :],
                        at_t[:, dc, :],
                        ht_slice(dc, tb),
                        start=(rnd == 0),
                        stop=(rnd == DC // 4 - 1),
                        tile_position=(0, 32 * j),
                    )

            def fold_and_evict(vb, tb):
                # reduce-and-fold the LoRA correction: one ordinary
                # K=128 matmul (bt4's zero rows mask the partial-group
                # padding), then evict, alternating DVE/ACT so banks
                # free ~2x faster
                ps = pss.pop((vb, tb))
                nc.tensor.matmul(
                    ps[:],
                    bt_t[:, vb * VBS:(vb + 1) * VBS],
                    pr_tiles[tb][:],
                    start=False,
                    stop=True,
                )
                ot = outp.tile([VBS, TBS], F16, name="ot", tag="ot")
                if tb % 2 == 0:
                    nc.vector.tensor_copy(ot[:], ps[:])
                else:
                    nc.scalar.copy(ot[:], ps[:])
                nc.scalar.dma_start(
                    outt[vb * VBS:(vb + 1) * VBS,
                         tb * TBS:(tb + 1) * TBS], ot[:],
                )

            N_RAMP = 3  # vbs interleaved with the ao rounds
            ramp_wt = [get_wt(vb) for vb in range(N_RAMP)]

            for g in range(2):
                tbs = (2 * g, 2 * g + 1)
                pas = {}
                for tb in tbs:
                    pas[tb] = aops.tile([P, TBS], F32, name="pa", tag="pa")
                for rnd in range(DC // 4):
                    for tb in tbs:
                        ao_round(pas[tb], rnd, tb)
                    for vb in range(N_RAMP):
                        for dc in range(4 * rnd, 4 * rnd + 4):
                            for tb in tbs:
                                base_mm(vb, ramp_wt[vb], dc, tb)
                for tb in tbs:
                    pr_t = const.tile([P, TBS], F16, name=f"pr{tb}")
                    nc.vector.tensor_copy(pr_t[:], pas[tb][:])
                    pr_tiles[tb] = pr_t
                for vb in range(N_RAMP):
                    for tb in tbs:
                        fold_and_evict(vb, tb)

            for vb in range(N_RAMP, VB):
                wt_t = get_wt(vb)
                for tbs in ((0, 1), (2, 3)):
                    for dc in range(DC):
                        for tb in tbs:
                            base_mm(vb, wt_t, dc, tb)
                    for tb in tbs:
                        fold_and_evict(vb, tb)
    nc.compile()
    return nc


def _prep_inputs(hidden_states, weight, lora_A, lora_B):
    w = np.asarray(weight, dtype=np.float16)
    # [core, vb, j, dc, p] -> [core, vb, p, dc, j]
    wtb_all = np.ascontiguousarray(
        w.reshape(NCORES, VB, VBS, DC, P).transpose(0, 1, 4, 3, 2)
    )
    htt = np.ascontiguousarray(np.asarray(hidden_states, dtype=np.float16).T)
    # A^T blocked [128, 32, 16], zero-padded to 32 columns
    att = np.asarray(lora_A, dtype=np.float16).T.reshape(DC, P, R).transpose(1, 0, 2)
    atz = np.zeros((P, DC, AZ), dtype=np.float16)
    atz[:, :, :R] = att
    # B^T slice replicated at 32-partition offsets, zeros elsewhere
    btt_all = (
        np.asarray(lora_B, dtype=np.float16).reshape(NCORES, VC, R)
        .transpose(0, 2, 1)
    )
    bt4_all = np.zeros((NCORES, P, VCP), dtype=np.float16)
    for j in range(4):
        bt4_all[:, 32 * j:32 * j + R, :] = btt_all
    return [
        {
            "wtb": wtb_all[c],
            "htt": htt,
            "atz": atz,
            "bt4": bt4_all[c],
        }
        for c in range(NCORES)
    ]


def run(hidden_states, weight, lora_A, lora_B, trace=False, **run_kwargs):
    in_maps = _prep_inputs(hidden_states, weight, lora_A, lora_B)
    nc = build_nc()
    res = run_bass_kernel_spmd(
        nc, in_maps, core_ids=list(range(NCORES)), trace=trace, **run_kwargs
    )
    out = np.empty((N_TOK, V), dtype=np.float32)
    for c in range(NCORES):
        out[:, c * VC:(c + 1) * VC] = res.results[c]["outt"].T.astype(np.float32)
    return out, res


def kernel(hidden_states, weight, lora_A, lora_B):
    out, _ = run(hidden_states, weight, lora_A, lora_B, trace=False)
    return out
